# revision 10
# baseline (speedup 1.0000x reference)
"""Multi-head attention TRN2 kernel: 8 cores = 4 batch x 2 head-groups.

Per core (b = core//2, g = core%2): attention for batch b, heads [8g, 8g+8),
producing the transposed partial output projection. Host sums the two
head-group partials per batch + bias.

v5 (scheduling rewrite of v4):
- Single software-pipelined loop over 16 (q-chunk, pair) phases. Each phase
  emits, per key-block slot: QK pair -> exp -> AV jobs of the PREVIOUS
  phase (slots 4+) -> one projection "piece". The PE queue is ordered so
  every instruction's deps are satisfied long before it is reached: the PE
  never idles, stays at full p-state, and the Act engine (exp, the
  co-bottleneck) is fed from ~15us instead of ~112us.
- All projections (K/V/Q/O) are uniform 1-accumulator pieces injected into
  the phase slots with emission deadlines; attention starts right after
  the minimal prologue (K pair0 + Q chunk0/pair0 + first V blocks).
- fp16 x/wq/wk everywhere on the Q/K path (verified: adds <1e-4 rel err vs
  f32r; fp16 store of Q/K dominates and was already present). Halves input
  DMA and lets x^T for K stay SBUF-resident.
- Normalize chain without DMA roundtrips: reciprocal straight off the PSUM
  rowsum row into partition 0, one partition_broadcast, two multiplies.
- Last-phase AV borrows a ps_s-tagged PSUM slot so it does not wait on the
  previous normalize (ps_u is single-buffered).

Layouts (per core, host-prepped):
  xq/xk : x^T     [1024 d, 2048 t] f16
  xv    : x^T     [1024 d, 2048 t] bf16
  wq/wk : W_g^T   [1024 d, 512 j]  f16
  wv    : W_g^T   [1024 d, 512 j]  bf16
  wo    : Wo_g^T  [512 c, 1024 j]  bf16
  out   : OUT^T partial [1024 j, 2048 t] f32
"""

import numpy as np
import ml_dtypes

D = 1024          # d_model
L = 2048          # sequence length
B = 4             # batch
HG = 512          # head-group width (8 heads x 64)
NCORES = 8
EXP_BIAS = -45.0  # softmax shift: exp(S-45); cancels in normalization

NT = 4            # token chunks of 512
TC = L // NT      # 512
NDB = D // 128    # 8 d-model blocks
NP = 4            # head pairs per group
NKB = L // 128    # 16 key blocks

_COMPILED = None
LAST_RESULT = None


def _build():
    import concourse.bacc as bacc
    import concourse.mybir as mybir
    import concourse.tile as tile

    f32 = mybir.dt.float32
    bf16 = mybir.dt.bfloat16
    f16 = mybir.dt.float16
    EXP = mybir.ActivationFunctionType.Exp
    ADD = mybir.AluOpType.add
    MUL = mybir.AluOpType.mult

    nc = bacc.Bacc()

    xq = nc.declare_dram_parameter("xq", [D, L], f16, isOutput=False)
    xk = nc.declare_dram_parameter("xk", [D, L], f16, isOutput=False)
    xv = nc.declare_dram_parameter("xv", [D, L], bf16, isOutput=False)
    wq = nc.declare_dram_parameter("wq", [D, HG], f16, isOutput=False)
    wk = nc.declare_dram_parameter("wk", [D, HG], f16, isOutput=False)
    wv = nc.declare_dram_parameter("wv", [D, HG], bf16, isOutput=False)
    wo = nc.declare_dram_parameter("wo", [HG, D], bf16, isOutput=False)
    bq = nc.declare_dram_parameter("bq", [HG], f32, isOutput=False)
    bv = nc.declare_dram_parameter("bv", [HG], f32, isOutput=False)
    out = nc.declare_dram_parameter("out", [D, L], f32, isOutput=True)

    out_v = out.rearrange("(ob p) (n t) -> ob p n t", p=128, t=TC)
    xq_v = xq.rearrange("(db p) (n t) -> p db n t", p=128, t=TC)
    xk_v = xk.rearrange("(db p) t -> p db t", p=128)
    xv_v = xv.rearrange("(db p) (n t) -> p db n t", p=128, t=TC)

    with tile.TileContext(nc) as tc:
        with tc.tile_pool(name="res", bufs=1) as res, tc.tile_pool(
            name="psum", bufs=1, space="PSUM"
        ) as psum, tc.tile_pool(name="work", bufs=1) as work:
            # ---- resident tiles ----
            kt_sb = res.tile([128, NP, L], f16)
            qt_sb = res.tile([128, NP, L], f16)
            xk_sb = res.tile([128, NDB, L], f16)
            wq_sb = res.tile([128, NDB, HG], f16)
            wk_sb = res.tile([128, NDB, HG], f16)
            wv_sb = res.tile([128, NDB, HG], bf16)
            wo_sb = res.tile([128, NP, D], bf16)
            # V in AV-stationary layout: per (kb, pair): [V_e, 1, V_o, 1]
            v_sb = res.tile([128, NKB, NP, 130], bf16)
            bq_sb = res.tile([128, NP], f32)
            bv_row = res.tile([1, HG], f32)
            bv_bc = res.tile([128, HG], f32)
            bias_exp = res.tile([128, 1], f32)
            dummy = res.tile([1, 1], f32)

            # ---- prologue DMAs, most-urgent first ----
            def load_xk_chunk(t):
                for db in range(NDB):
                    nc.sync.dma_start(
                        out=xk_sb[:, db, t * TC: (t + 1) * TC],
                        in_=xk_v[:, db, t * TC: (t + 1) * TC])

            load_xk_chunk(0)
            nc.sync.dma_start(out=wk_sb[:, 0:4], in_=wk.rearrange(
                "(db p) j -> p db j", p=128)[:, 0:4])
            nc.sync.dma_start(out=wk_sb[:, 4:8], in_=wk.rearrange(
                "(db p) j -> p db j", p=128)[:, 4:8])
            nc.sync.dma_start(out=wq_sb[:, 0:4], in_=wq.rearrange(
                "(db p) j -> p db j", p=128)[:, 0:4])
            nc.sync.dma_start(out=wq_sb[:, 4:8], in_=wq.rearrange(
                "(db p) j -> p db j", p=128)[:, 4:8])
            nc.sync.dma_start(
                out=bq_sb[:], in_=bq.rearrange("(jb p) -> p jb", p=128))
            nc.sync.dma_start(
                out=bv_row[:], in_=bv.rearrange("(o j) -> o j", o=1))

            def load_xq(t):
                xt = work.tile([128, NDB, TC], f16, name="xqt", tag="xqt",
                               bufs=2)
                nc.sync.dma_start(out=xt[:, 0:4], in_=xq_v[:, 0:4, t])
                nc.sync.dma_start(out=xt[:, 4:8], in_=xq_v[:, 4:8, t])
                return xt

            def load_xv(t):
                xt = work.tile([128, NDB, TC], bf16, name="xvt", tag="xvt",
                               bufs=2)
                nc.sync.dma_start(out=xt[:, 0:4], in_=xv_v[:, 0:4, t])
                nc.sync.dma_start(out=xt[:, 4:8], in_=xv_v[:, 4:8, t])
                return xt

            xq_t = [load_xq(0), None, None, None]
            load_xk_chunk(1)
            xv_t = [load_xv(0), None, None, None]
            nc.sync.dma_start(out=wv_sb[:, 0:4], in_=wv.rearrange(
                "(db p) j -> p db j", p=128)[:, 0:4])
            nc.sync.dma_start(out=wv_sb[:, 4:8], in_=wv.rearrange(
                "(db p) j -> p db j", p=128)[:, 4:8])
            load_xk_chunk(2)
            load_xk_chunk(3)
            xv_t[1] = load_xv(1)
            nc.sync.dma_start(out=wo_sb[:, 0:2], in_=wo.rearrange(
                "(cb p) j -> p cb j", p=128)[:, 0:2])
            nc.sync.dma_start(out=wo_sb[:, 2:4], in_=wo.rearrange(
                "(cb p) j -> p cb j", p=128)[:, 2:4])

            nc.gpsimd.partition_broadcast(bv_bc[:], bv_row[:], channels=128)
            nc.vector.memset(bias_exp[:], EXP_BIAS)
            nc.vector.memset(v_sb[:, :, :, 64:65], 1.0)
            nc.vector.memset(v_sb[:, :, :, 129:130], 1.0)
            # pull the Exp table into the Act engine off the critical path
            nc.scalar.activation(dummy[:], bias_exp[0:1, 0:1], EXP,
                                 bias=bias_exp[0:1, :], scale=1.0)

            # ---- projection pieces (uniform 1-accumulator units) ----
            def kpiece(t, jb):
                ps = psum.tile([128, TC], f32, name="acc", tag="accu", bufs=2)
                for db in range(NDB):
                    nc.tensor.matmul(
                        ps[:],
                        wk_sb[:, db, jb * 128: (jb + 1) * 128],
                        xk_sb[:, db, t * TC: (t + 1) * TC],
                        start=(db == 0), stop=(db == NDB - 1),
                    )
                nc.vector.tensor_copy(kt_sb[:, jb, t * TC: (t + 1) * TC], ps[:])

            def qpiece(t, jb):
                ps = psum.tile([128, TC], f32, name="acc", tag="accu", bufs=2)
                for db in range(NDB):
                    nc.tensor.matmul(
                        ps[:],
                        wq_sb[:, db, jb * 128: (jb + 1) * 128],
                        xq_t[t][:, db, :],
                        start=(db == 0), stop=(db == NDB - 1),
                    )
                nc.vector.tensor_scalar_add(
                    qt_sb[:, jb, t * TC: (t + 1) * TC], ps[:],
                    bq_sb[:, jb: jb + 1])

            def vpiece(t, tb):
                kb = t * 4 + tb
                ps = psum.tile([128, HG], f32, name="acc", tag="accu", bufs=2)
                for db in range(NDB):
                    nc.tensor.matmul(
                        ps[:],
                        xv_t[t][:, db, tb * 128: (tb + 1) * 128],
                        wv_sb[:, db, :],
                        start=(db == 0), stop=(db == NDB - 1),
                    )
                for p in range(NP):
                    nc.vector.tensor_tensor(
                        out=v_sb[:, kb, p, 0:64],
                        in0=ps[:, p * 128: p * 128 + 64],
                        in1=bv_bc[:, p * 128: p * 128 + 64], op=ADD)
                    nc.vector.tensor_tensor(
                        out=v_sb[:, kb, p, 65:129],
                        in0=ps[:, p * 128 + 64: p * 128 + 128],
                        in1=bv_bc[:, p * 128 + 64: p * 128 + 128], op=ADD)

            ct_tiles = {}

            def opiece(q_, ob):
                ps = psum.tile([128, TC], f32, name="acc", tag="accu", bufs=2)
                ct_t = ct_tiles[q_]
                for p in range(NP):
                    nc.tensor.matmul(
                        ps[:],
                        wo_sb[:, p, ob * 128: (ob + 1) * 128],
                        ct_t[:, p, :],
                        start=(p == 0), stop=(p == NP - 1),
                    )
                o_sb = work.tile([128, TC], f32, name="o_sb", tag="osb",
                                 bufs=2)
                nc.vector.tensor_copy(o_sb[:], ps[:])
                nc.sync.dma_start(out=out_v[ob, :, q_], in_=o_sb[:])

            def normalize(q_, p_, u_sb):
                if p_ == 0:
                    ct_tiles[q_] = work.tile(
                        [128, NP, TC], bf16, name="ct", tag="ct", bufs=2)
                ct_t = ct_tiles[q_]
                # single-lane DVE reciprocal is ~6ns/elem: spread the 1024
                # rowsums over 128 lanes via a DMA roundtrip instead
                r128 = work.tile([128, 8], f32, name="r128", tag="r128",
                                 bufs=2)
                nc.sync.dma_start(out=r128[:, 0:4], in_=u_sb[64:65, 0, :])
                nc.sync.dma_start(out=r128[:, 4:8], in_=u_sb[64:65, 1, :])
                nc.vector.reciprocal(r128[:], r128[:])
                rr = work.tile([1, 2, TC], f32, name="rr", tag="rr", bufs=1)
                nc.sync.dma_start(out=rr[:, 0, :], in_=r128[:, 0:4])
                nc.sync.dma_start(out=rr[:, 1, :], in_=r128[:, 4:8])
                rb = work.tile([64, 2, TC], f32, name="rb", tag="rb", bufs=1)
                nc.gpsimd.partition_broadcast(rb[:], rr[:], channels=64)
                nc.vector.tensor_tensor(
                    out=ct_t[0:64, p_, :], in0=u_sb[0:64, 0, :],
                    in1=rb[:, 0, :], op=MUL)
                cto = work.tile([64, TC], bf16, name="cto", tag="cto", bufs=1)
                nc.vector.tensor_tensor(
                    out=cto[:], in0=u_sb[0:64, 1, :], in1=rb[:, 1, :], op=MUL)
                nc.sync.dma_start(out=ct_t[64:128, p_, :], in_=cto[:])

            # ---- prologue pieces: minimum to unblock phase 0 ----
            kpiece(0, 0)
            qpiece(0, 0)

            # ---- per-phase piece schedule ----
            # Emission deadlines: kt pair p by phase p; qt (c, jb) by phase
            # 4c+jb; v_sb[kb] before the AV job for kb pops (slot kb+AV_LAG
            # of its phase); ct(q) O-projection spread over phases 4q+4..+7.
            def loadxq(t):
                xq_t[t] = load_xq(t)

            def loadxv(t):
                xv_t[t] = load_xv(t)

            def v03_then_loadxv2():
                vpiece(0, 3)
                loadxv(2)

            def v13_then_loadxv3():
                vpiece(1, 3)
                loadxv(3)

            def v33_then_loadxq1():
                vpiece(3, 3)
                loadxq(1)

            sched = [[] for _ in range(16)]
            sched[0] = [lambda: kpiece(1, 0), lambda: vpiece(0, 0),
                        lambda: vpiece(0, 1), lambda: vpiece(0, 2),
                        lambda: kpiece(2, 0), v03_then_loadxv2,
                        lambda: vpiece(1, 0), lambda: vpiece(1, 1),
                        lambda: kpiece(3, 0), lambda: vpiece(1, 2),
                        v13_then_loadxv3] + \
                [(lambda t=t: kpiece(t, 1)) for t in range(NT)] + \
                [lambda: qpiece(0, 1)]
            sched[1] = [(lambda tb=tb: vpiece(2, tb)) for tb in range(4)] + \
                [(lambda tb=tb: vpiece(3, tb)) for tb in range(3)] + \
                [v33_then_loadxq1] + \
                [(lambda t=t: kpiece(t, 2)) for t in range(NT)] + \
                [lambda: qpiece(0, 2)]
            sched[2] = [(lambda t=t: kpiece(t, 3)) for t in range(NT)] + \
                [lambda: qpiece(0, 3)]
            for c in (1, 2, 3):
                for jb in range(4):
                    sched[4 * c + jb - 1].append(
                        lambda c=c, jb=jb: qpiece(c, jb))
            sched[4].append(lambda: loadxq(2))
            sched[8].append(lambda: loadxq(3))
            # opiece(q, 0/1) must trail normalize(q, p3), which pops at
            # slot AV_LAG-1 of phase 4q+4 (min_slot below defers them).
            for q_ in range(3):
                for ob in range(NDB):
                    sched[4 * q_ + 4 + ob // 2].append(
                        (lambda q_=q_, ob=ob: opiece(q_, ob),
                         10 if ob < 2 else 0))

            # ---- main loop: 16 (q-chunk, pair) phases ----
            # Per slot: QK pair -> exp -> pop trailing AV jobs -> pieces.
            # AV trails exp by AV_LAG slots; ps_u is drained to SBUF right
            # after the last AV job so the single PSUM slot recycles fast.
            av_queue = []

            def pop_av():
                st, kb = av_queue.pop(0)
                if kb == 0:
                    st["psu"] = psum.tile([128, 2, TC], f32, name="ps_u",
                                          tag="psu", bufs=1)
                psu = st["psu"]
                nc.tensor.matmul(
                    psu[0:65, 0, :], v_sb[:, kb, st["p"], 0:65],
                    st["pt"][:, kb, 0, :],
                    start=(kb == 0), stop=(kb == NKB - 1))
                nc.tensor.matmul(
                    psu[0:65, 1, :], v_sb[:, kb, st["p"], 65:130],
                    st["pt"][:, kb, 1, :],
                    start=(kb == 0), stop=(kb == NKB - 1))
                if kb == NKB - 1:
                    u_sb = work.tile([65, 2, TC], f32, name="u_sb",
                                     tag="usb", bufs=1)
                    nc.vector.tensor_copy(u_sb[:], psu[0:65, :, :])
                    normalize(st["q"], st["p"], u_sb)

            for ph in range(16):
                q, p = ph // 4, ph % 4
                pieces = [e if isinstance(e, tuple) else (e, 0)
                          for e in sched[ph]]
                assert len(pieces) <= NKB
                pieces_done = 0
                pt = work.tile([128, NKB, 2, TC], bf16, name="pt", tag="pt",
                               bufs=1)
                st = {"pt": pt, "q": q, "p": p, "psu": None}
                for kb in range(NKB):
                    ps_qk = psum.tile([128, 2, TC], f32, name="ps_s",
                                      tag="ps_s", bufs=2)
                    nc.tensor.matmul(
                        ps_qk[:, 0, :],
                        kt_sb[0:64, p, kb * 128: (kb + 1) * 128],
                        qt_sb[0:64, p, q * TC: (q + 1) * TC],
                        start=True, stop=True)
                    nc.tensor.matmul(
                        ps_qk[:, 1, :],
                        kt_sb[64:128, p, kb * 128: (kb + 1) * 128],
                        qt_sb[64:128, p, q * TC: (q + 1) * TC],
                        start=True, stop=True)
                    nc.scalar.activation(
                        pt[:, kb, :, :], ps_qk[:], EXP,
                        bias=bias_exp[:], scale=1.0)
                    av_queue.append((st, kb))
                    av_lag = 9 if ph <= 1 else 5
                    while len(av_queue) > av_lag:
                        pop_av()
                    while pieces_done < min(len(pieces), kb + 1) and \
                            pieces[pieces_done][1] <= kb:
                        pieces[pieces_done][0]()
                        pieces_done += 1

            # ---- tail: drain AV queue, last O-projection ----
            while av_queue:
                pop_av()
            for ob in range(NDB):
                opiece(3, ob)

    nc.compile()
    return nc


def _get_compiled():
    global _COMPILED
    if _COMPILED is None:
        _COMPILED = _build()
    return _COMPILED


def kernel(q, k, v, Wq, bq, Wk, bk, Wv, bv, Wo, bo):
    global LAST_RESULT
    from concourse.bass_utils import run_bass_kernel_spmd

    nc = _get_compiled()

    q = np.asarray(q, dtype=np.float32)
    k = np.asarray(k, dtype=np.float32)
    v = np.asarray(v, dtype=np.float32)
    Wq = np.asarray(Wq, dtype=np.float32)
    Wk = np.asarray(Wk, dtype=np.float32)
    Wv = np.asarray(Wv, dtype=np.float32)
    Wo = np.asarray(Wo, dtype=np.float32)
    bq = np.asarray(bq, dtype=np.float32)
    bv = np.asarray(bv, dtype=np.float32)
    bo = np.asarray(bo, dtype=np.float32)

    xT = {}
    for b in range(B):
        xT[("q", b)] = np.ascontiguousarray(q[b].T).astype(np.float16)
        xT[("k", b)] = np.ascontiguousarray(k[b].T).astype(np.float16)
        xT[("v", b)] = np.ascontiguousarray(v[b].T).astype(ml_dtypes.bfloat16)

    wqT = [np.ascontiguousarray(Wq[g * HG: (g + 1) * HG, :].T).astype(
        np.float16) for g in range(2)]
    wkT = [np.ascontiguousarray(Wk[g * HG: (g + 1) * HG, :].T).astype(
        np.float16) for g in range(2)]
    wvT = [np.ascontiguousarray(Wv[g * HG: (g + 1) * HG, :].T).astype(
        ml_dtypes.bfloat16) for g in range(2)]
    woT = [np.ascontiguousarray(Wo[:, g * HG: (g + 1) * HG].T).astype(
        ml_dtypes.bfloat16) for g in range(2)]
    bqg = [np.ascontiguousarray(bq[g * HG: (g + 1) * HG]) for g in range(2)]
    bvg = [np.ascontiguousarray(bv[g * HG: (g + 1) * HG]) for g in range(2)]

    in_maps = []
    for core in range(NCORES):
        b, g = core // 2, core % 2
        in_maps.append(
            {
                "xq": xT[("q", b)],
                "xk": xT[("k", b)],
                "xv": xT[("v", b)],
                "wq": wqT[g],
                "wk": wkT[g],
                "wv": wvT[g],
                "wo": woT[g],
                "bq": bqg[g],
                "bv": bvg[g],
            }
        )

    res = run_bass_kernel_spmd(nc, in_maps, core_ids=list(range(NCORES)))
    LAST_RESULT = res

    outp = np.empty((B, L, D), dtype=np.float32)
    for b in range(B):
        acc = res.results[2 * b]["out"].T + res.results[2 * b + 1]["out"].T
        outp[b] = acc + bo
    return outp


# revision 13
# speedup vs baseline: 1.0114x; 1.0114x over previous
"""Multi-head attention TRN2 kernel: 8 cores = 4 batch x 2 head-groups.

Per core (b = core//2, g = core%2): attention for batch b, heads [8g, 8g+8),
producing the transposed partial output projection. Host sums the two
head-group partials per batch + bias.

v5 (scheduling rewrite of v4):
- Single software-pipelined loop over 16 (q-chunk, pair) phases. Each phase
  emits, per key-block slot: QK pair -> exp -> AV jobs of the PREVIOUS
  phase (slots 4+) -> one projection "piece". The PE queue is ordered so
  every instruction's deps are satisfied long before it is reached: the PE
  never idles, stays at full p-state, and the Act engine (exp, the
  co-bottleneck) is fed from ~15us instead of ~112us.
- All projections (K/V/Q/O) are uniform 1-accumulator pieces injected into
  the phase slots with emission deadlines; attention starts right after
  the minimal prologue (K pair0 + Q chunk0/pair0 + first V blocks).
- fp16 x/wq/wk everywhere on the Q/K path (verified: adds <1e-4 rel err vs
  f32r; fp16 store of Q/K dominates and was already present). Halves input
  DMA and lets x^T for K stay SBUF-resident.
- Normalize chain without DMA roundtrips: reciprocal straight off the PSUM
  rowsum row into partition 0, one partition_broadcast, two multiplies.
- Last-phase AV borrows a ps_s-tagged PSUM slot so it does not wait on the
  previous normalize (ps_u is single-buffered).

Layouts (per core, host-prepped):
  xq/xk : x^T     [1024 d, 2048 t] f16
  xv    : x^T     [1024 d, 2048 t] bf16
  wq/wk : W_g^T   [1024 d, 512 j]  f16
  wv    : W_g^T   [1024 d, 512 j]  bf16
  wo    : Wo_g^T  [512 c, 1024 j]  bf16
  out   : OUT^T partial [1024 j, 2048 t] f32
"""

import numpy as np
import ml_dtypes

D = 1024          # d_model
L = 2048          # sequence length
B = 4             # batch
HG = 512          # head-group width (8 heads x 64)
NCORES = 8
EXP_BIAS = -45.0  # softmax shift: exp(S-45); cancels in normalization

NT = 4            # token chunks of 512
TC = L // NT      # 512
NDB = D // 128    # 8 d-model blocks
NP = 4            # head pairs per group
NKB = L // 128    # 16 key blocks

_COMPILED = None
LAST_RESULT = None


def _build():
    import concourse.bacc as bacc
    import concourse.mybir as mybir
    import concourse.tile as tile

    f32 = mybir.dt.float32
    bf16 = mybir.dt.bfloat16
    f16 = mybir.dt.float16
    EXP = mybir.ActivationFunctionType.Exp
    ADD = mybir.AluOpType.add
    MUL = mybir.AluOpType.mult

    nc = bacc.Bacc()

    xq = nc.declare_dram_parameter("xq", [D, L], f16, isOutput=False)
    xk = nc.declare_dram_parameter("xk", [D, L], f16, isOutput=False)
    xv = nc.declare_dram_parameter("xv", [D, L], bf16, isOutput=False)
    wq = nc.declare_dram_parameter("wq", [D, HG], f16, isOutput=False)
    wk = nc.declare_dram_parameter("wk", [D, HG], f16, isOutput=False)
    wv = nc.declare_dram_parameter("wv", [D, HG], bf16, isOutput=False)
    wo = nc.declare_dram_parameter("wo", [HG, D], bf16, isOutput=False)
    bq = nc.declare_dram_parameter("bq", [HG], f32, isOutput=False)
    bv = nc.declare_dram_parameter("bv", [HG], f32, isOutput=False)
    out = nc.declare_dram_parameter("out", [D, L], f32, isOutput=True)

    out_v = out.rearrange("(ob p) (n t) -> ob p n t", p=128, t=TC)
    xq_v = xq.rearrange("(db p) (n t) -> p db n t", p=128, t=TC)
    xk_v = xk.rearrange("(db p) t -> p db t", p=128)
    xv_v = xv.rearrange("(db p) (n t) -> p db n t", p=128, t=TC)

    with tile.TileContext(nc) as tc:
        with tc.tile_pool(name="res", bufs=1) as res, tc.tile_pool(
            name="psum", bufs=1, space="PSUM"
        ) as psum, tc.tile_pool(name="work", bufs=1) as work:
            # ---- resident tiles ----
            kt_sb = res.tile([128, NP, L], f16)
            qt_sb = res.tile([128, NP, L], f16)
            xk_sb = res.tile([128, NDB, L], f16)
            wq_sb = res.tile([128, NDB, HG], f16)
            wk_sb = res.tile([128, NDB, HG], f16)
            wv_sb = res.tile([128, NDB, HG], bf16)
            wo_sb = res.tile([128, NP, D], bf16)
            # V in AV-stationary layout: per (kb, pair): [V_e, 1, V_o, 1]
            v_sb = res.tile([128, NKB, NP, 130], bf16)
            bq_sb = res.tile([128, NP], f32)
            bv_row = res.tile([1, HG], f32)
            bv_bc = res.tile([128, HG], f32)
            bias_exp = res.tile([128, 1], f32)
            dummy = res.tile([1, 1], f32)

            # ---- prologue DMAs, most-urgent first ----
            for db in range(NDB):
                nc.sync.dma_start(out=xk_sb[:, db], in_=xk_v[:, db])
            nc.sync.dma_start(out=wk_sb[:], in_=wk.rearrange(
                "(db p) j -> p db j", p=128))
            nc.sync.dma_start(out=wq_sb[:], in_=wq.rearrange(
                "(db p) j -> p db j", p=128))
            nc.sync.dma_start(
                out=bq_sb[:], in_=bq.rearrange("(jb p) -> p jb", p=128))
            nc.sync.dma_start(
                out=bv_row[:], in_=bv.rearrange("(o j) -> o j", o=1))

            def load_xq(t):
                xt = work.tile([128, NDB, TC], f16, name="xqt", tag="xqt",
                               bufs=2)
                nc.sync.dma_start(out=xt[:], in_=xq_v[:, :, t])
                return xt

            def load_xv(t):
                xt = work.tile([128, NDB, TC], bf16, name="xvt", tag="xvt",
                               bufs=2)
                nc.sync.dma_start(out=xt[:], in_=xv_v[:, :, t])
                return xt

            xq_t = [load_xq(0), None, None, None]
            xv_t = [load_xv(0), None, None, None]
            nc.sync.dma_start(out=wv_sb[:], in_=wv.rearrange(
                "(db p) j -> p db j", p=128))
            xv_t[1] = load_xv(1)
            nc.sync.dma_start(out=wo_sb[:], in_=wo.rearrange(
                "(cb p) j -> p cb j", p=128))

            nc.gpsimd.partition_broadcast(bv_bc[:], bv_row[:], channels=128)
            nc.vector.memset(bias_exp[:], EXP_BIAS)
            nc.vector.memset(v_sb[:, :, :, 64:65], 1.0)
            nc.vector.memset(v_sb[:, :, :, 129:130], 1.0)
            # pull the Exp table into the Act engine off the critical path
            nc.scalar.activation(dummy[:], bias_exp[0:1, 0:1], EXP,
                                 bias=bias_exp[0:1, :], scale=1.0)

            # ---- projection pieces (uniform 1-accumulator units) ----
            def kpiece(t, jb):
                ps = psum.tile([128, TC], f32, name="acc", tag="accu", bufs=2)
                for db in range(NDB):
                    nc.tensor.matmul(
                        ps[:],
                        wk_sb[:, db, jb * 128: (jb + 1) * 128],
                        xk_sb[:, db, t * TC: (t + 1) * TC],
                        start=(db == 0), stop=(db == NDB - 1),
                    )
                nc.vector.tensor_copy(kt_sb[:, jb, t * TC: (t + 1) * TC], ps[:])

            def qpiece(t, jb):
                ps = psum.tile([128, TC], f32, name="acc", tag="accu", bufs=2)
                for db in range(NDB):
                    nc.tensor.matmul(
                        ps[:],
                        wq_sb[:, db, jb * 128: (jb + 1) * 128],
                        xq_t[t][:, db, :],
                        start=(db == 0), stop=(db == NDB - 1),
                    )
                nc.vector.tensor_scalar_add(
                    qt_sb[:, jb, t * TC: (t + 1) * TC], ps[:],
                    bq_sb[:, jb: jb + 1])

            def vpiece(t, tb):
                kb = t * 4 + tb
                ps = psum.tile([128, HG], f32, name="acc", tag="accu", bufs=2)
                for db in range(NDB):
                    nc.tensor.matmul(
                        ps[:],
                        xv_t[t][:, db, tb * 128: (tb + 1) * 128],
                        wv_sb[:, db, :],
                        start=(db == 0), stop=(db == NDB - 1),
                    )
                for p in range(NP):
                    nc.vector.tensor_tensor(
                        out=v_sb[:, kb, p, 0:64],
                        in0=ps[:, p * 128: p * 128 + 64],
                        in1=bv_bc[:, p * 128: p * 128 + 64], op=ADD)
                    nc.vector.tensor_tensor(
                        out=v_sb[:, kb, p, 65:129],
                        in0=ps[:, p * 128 + 64: p * 128 + 128],
                        in1=bv_bc[:, p * 128 + 64: p * 128 + 128], op=ADD)

            ct_tiles = {}

            def opiece(q_, ob):
                ps = psum.tile([128, TC], f32, name="acc", tag="accu", bufs=2)
                ct_t = ct_tiles[q_]
                for p in range(NP):
                    nc.tensor.matmul(
                        ps[:],
                        wo_sb[:, p, ob * 128: (ob + 1) * 128],
                        ct_t[:, p, :],
                        start=(p == 0), stop=(p == NP - 1),
                    )
                o_sb = work.tile([128, TC], f32, name="o_sb", tag="osb",
                                 bufs=2)
                nc.vector.tensor_copy(o_sb[:], ps[:])
                nc.sync.dma_start(out=out_v[ob, :, q_], in_=o_sb[:])

            def normalize(q_, p_, u_sb):
                if p_ == 0:
                    ct_tiles[q_] = work.tile(
                        [128, NP, TC], bf16, name="ct", tag="ct", bufs=2)
                ct_t = ct_tiles[q_]
                # single-lane DVE reciprocal is ~6ns/elem: spread the 1024
                # rowsums over 128 lanes via a DMA roundtrip instead
                r128 = work.tile([128, 8], f32, name="r128", tag="r128",
                                 bufs=2)
                nc.sync.dma_start(out=r128[:], in_=u_sb[64:65, :, :])
                nc.vector.reciprocal(r128[:], r128[:])
                rr = work.tile([1, 2, TC], f32, name="rr", tag="rr", bufs=1)
                nc.sync.dma_start(out=rr[:], in_=r128[:])
                rb = work.tile([64, 2, TC], f32, name="rb", tag="rb", bufs=1)
                nc.gpsimd.partition_broadcast(rb[:], rr[:], channels=64)
                nc.vector.tensor_tensor(
                    out=ct_t[0:64, p_, :], in0=u_sb[0:64, 0, :],
                    in1=rb[:, 0, :], op=MUL)
                cto = work.tile([64, TC], bf16, name="cto", tag="cto", bufs=1)
                nc.vector.tensor_tensor(
                    out=cto[:], in0=u_sb[0:64, 1, :], in1=rb[:, 1, :], op=MUL)
                nc.sync.dma_start(out=ct_t[64:128, p_, :], in_=cto[:])

            # ---- prologue pieces: minimum to unblock phase 0 ----
            kpiece(0, 0)
            qpiece(0, 0)

            # ---- per-phase piece schedule ----
            # Emission deadlines: kt pair p by phase p; qt (c, jb) by phase
            # 4c+jb; v_sb[kb] before the AV job for kb pops (slot kb+AV_LAG
            # of its phase); ct(q) O-projection spread over phases 4q+4..+7.
            def loadxq(t):
                xq_t[t] = load_xq(t)

            def loadxv(t):
                xv_t[t] = load_xv(t)

            def v03_then_loadxv2():
                vpiece(0, 3)
                loadxv(2)

            def v13_then_loadxv3():
                vpiece(1, 3)
                loadxv(3)

            def v33_then_loadxq1():
                vpiece(3, 3)
                loadxq(1)

            sched = [[] for _ in range(16)]
            sched[0] = [lambda: kpiece(1, 0), lambda: vpiece(0, 0),
                        lambda: vpiece(0, 1), lambda: vpiece(0, 2),
                        lambda: kpiece(2, 0), v03_then_loadxv2,
                        lambda: vpiece(1, 0), lambda: vpiece(1, 1),
                        lambda: kpiece(3, 0), lambda: vpiece(1, 2),
                        v13_then_loadxv3] + \
                [(lambda t=t: kpiece(t, 1)) for t in range(NT)] + \
                [lambda: qpiece(0, 1)]
            sched[1] = [(lambda tb=tb: vpiece(2, tb)) for tb in range(4)] + \
                [(lambda tb=tb: vpiece(3, tb)) for tb in range(3)] + \
                [v33_then_loadxq1] + \
                [(lambda t=t: kpiece(t, 2)) for t in range(NT)] + \
                [lambda: qpiece(0, 2)]
            sched[2] = [(lambda t=t: kpiece(t, 3)) for t in range(NT)] + \
                [lambda: qpiece(0, 3)]
            for c in (1, 2, 3):
                for jb in range(4):
                    sched[4 * c + jb - 1].append(
                        lambda c=c, jb=jb: qpiece(c, jb))
            sched[4].append(lambda: loadxq(2))
            sched[8].append(lambda: loadxq(3))
            # opiece(q, 0/1) must trail normalize(q, p3), which pops at
            # slot AV_LAG-1 of phase 4q+4; min_slots also space the pieces
            # out so exp is never starved by back-to-back projections.
            OSLOT = [10, 13, 4, 9, 4, 9, 4, 9]
            for q_ in range(3):
                for ob in range(NDB):
                    sched[4 * q_ + 4 + ob // 2].append(
                        (lambda q_=q_, ob=ob: opiece(q_, ob), OSLOT[ob]))

            # ---- main loop: 16 (q-chunk, pair) phases ----
            # Per slot: QK pair -> exp -> pop trailing AV jobs -> pieces.
            # AV trails exp by AV_LAG slots; ps_u is drained to SBUF right
            # after the last AV job so the single PSUM slot recycles fast.
            av_queue = []

            def pop_av():
                st, kb = av_queue.pop(0)
                if kb == 0:
                    st["psu"] = psum.tile([128, 2, TC], f32, name="ps_u",
                                          tag="psu", bufs=1)
                psu = st["psu"]
                nc.tensor.matmul(
                    psu[0:65, 0, :], v_sb[:, kb, st["p"], 0:65],
                    st["pt"][:, kb, 0, :],
                    start=(kb == 0), stop=(kb == NKB - 1))
                nc.tensor.matmul(
                    psu[0:65, 1, :], v_sb[:, kb, st["p"], 65:130],
                    st["pt"][:, kb, 1, :],
                    start=(kb == 0), stop=(kb == NKB - 1))
                if kb == NKB - 1:
                    u_sb = work.tile([65, 2, TC], f32, name="u_sb",
                                     tag="usb", bufs=1)
                    nc.vector.tensor_copy(u_sb[:], psu[0:65, :, :])
                    normalize(st["q"], st["p"], u_sb)

            for ph in range(16):
                q, p = ph // 4, ph % 4
                pieces = [e if isinstance(e, tuple) else (e, 0)
                          for e in sched[ph]]
                assert len(pieces) <= NKB
                pieces_done = 0
                pt = work.tile([128, NKB, 2, TC], bf16, name="pt", tag="pt",
                               bufs=1)
                st = {"pt": pt, "q": q, "p": p, "psu": None}
                for kb in range(NKB):
                    ps_qk = psum.tile([128, 2, TC], f32, name="ps_s",
                                      tag="ps_s", bufs=2)
                    nc.tensor.matmul(
                        ps_qk[:, 0, :],
                        kt_sb[0:64, p, kb * 128: (kb + 1) * 128],
                        qt_sb[0:64, p, q * TC: (q + 1) * TC],
                        start=True, stop=True)
                    nc.tensor.matmul(
                        ps_qk[:, 1, :],
                        kt_sb[64:128, p, kb * 128: (kb + 1) * 128],
                        qt_sb[64:128, p, q * TC: (q + 1) * TC],
                        start=True, stop=True)
                    nc.scalar.activation(
                        pt[:, kb, :, :], ps_qk[:], EXP,
                        bias=bias_exp[:], scale=1.0)
                    av_queue.append((st, kb))
                    av_lag = 9 if ph <= 1 else 5
                    while len(av_queue) > av_lag:
                        pop_av()
                    while pieces_done < min(len(pieces), kb + 1) and \
                            pieces[pieces_done][1] <= kb:
                        pieces[pieces_done][0]()
                        pieces_done += 1

            # ---- tail: drain AV queue, last O-projection ----
            # Prefix p0-p2 of the first four O-groups while the last
            # normalize chain is still in flight (only p3 depends on it);
            # ps_s slots are free once the last exp has drained.
            while av_queue:
                pop_av()

            def o_group(ob, ps):
                for p in range(NP):
                    nc.tensor.matmul(
                        ps,
                        wo_sb[:, p, ob * 128: (ob + 1) * 128],
                        ct_tiles[3][:, p, :],
                        start=(p == 0), stop=(p == NP - 1),
                    )
                o_sb = work.tile([128, TC], f32, name="o_sb", tag="osb",
                                 bufs=2)
                nc.vector.tensor_copy(o_sb[:], ps)
                nc.sync.dma_start(out=out_v[ob, :, 3], in_=o_sb[:])

            pre = []
            for ob in range(4):
                if ob < 2:
                    ps = psum.tile([128, TC], f32, name="acc", tag="accu",
                                   bufs=2)[:]
                else:
                    ps = psum.tile([128, 2, TC], f32, name="ps_o",
                                   tag="ps_s", bufs=2)[:, 0, :]
                for p in range(NP - 1):
                    nc.tensor.matmul(
                        ps,
                        wo_sb[:, p, ob * 128: (ob + 1) * 128],
                        ct_tiles[3][:, p, :],
                        start=(p == 0), stop=False,
                    )
                pre.append(ps)
            for ob in range(4):
                ps = pre[ob]
                nc.tensor.matmul(
                    ps,
                    wo_sb[:, 3, ob * 128: (ob + 1) * 128],
                    ct_tiles[3][:, 3, :],
                    start=False, stop=True,
                )
                o_sb = work.tile([128, TC], f32, name="o_sb", tag="osb",
                                 bufs=2)
                nc.vector.tensor_copy(o_sb[:], ps)
                nc.sync.dma_start(out=out_v[ob, :, 3], in_=o_sb[:])
            for ob in range(4, NDB):
                opiece(3, ob)

    nc.compile()
    return nc


def _get_compiled():
    global _COMPILED
    if _COMPILED is None:
        _COMPILED = _build()
    return _COMPILED


def kernel(q, k, v, Wq, bq, Wk, bk, Wv, bv, Wo, bo):
    global LAST_RESULT
    from concourse.bass_utils import run_bass_kernel_spmd

    nc = _get_compiled()

    q = np.asarray(q, dtype=np.float32)
    k = np.asarray(k, dtype=np.float32)
    v = np.asarray(v, dtype=np.float32)
    Wq = np.asarray(Wq, dtype=np.float32)
    Wk = np.asarray(Wk, dtype=np.float32)
    Wv = np.asarray(Wv, dtype=np.float32)
    Wo = np.asarray(Wo, dtype=np.float32)
    bq = np.asarray(bq, dtype=np.float32)
    bv = np.asarray(bv, dtype=np.float32)
    bo = np.asarray(bo, dtype=np.float32)

    xT = {}
    for b in range(B):
        xT[("q", b)] = np.ascontiguousarray(q[b].T).astype(np.float16)
        xT[("k", b)] = np.ascontiguousarray(k[b].T).astype(np.float16)
        xT[("v", b)] = np.ascontiguousarray(v[b].T).astype(ml_dtypes.bfloat16)

    wqT = [np.ascontiguousarray(Wq[g * HG: (g + 1) * HG, :].T).astype(
        np.float16) for g in range(2)]
    wkT = [np.ascontiguousarray(Wk[g * HG: (g + 1) * HG, :].T).astype(
        np.float16) for g in range(2)]
    wvT = [np.ascontiguousarray(Wv[g * HG: (g + 1) * HG, :].T).astype(
        ml_dtypes.bfloat16) for g in range(2)]
    woT = [np.ascontiguousarray(Wo[:, g * HG: (g + 1) * HG].T).astype(
        ml_dtypes.bfloat16) for g in range(2)]
    bqg = [np.ascontiguousarray(bq[g * HG: (g + 1) * HG]) for g in range(2)]
    bvg = [np.ascontiguousarray(bv[g * HG: (g + 1) * HG]) for g in range(2)]

    in_maps = []
    for core in range(NCORES):
        b, g = core // 2, core % 2
        in_maps.append(
            {
                "xq": xT[("q", b)],
                "xk": xT[("k", b)],
                "xv": xT[("v", b)],
                "wq": wqT[g],
                "wk": wkT[g],
                "wv": wvT[g],
                "wo": woT[g],
                "bq": bqg[g],
                "bv": bvg[g],
            }
        )

    res = run_bass_kernel_spmd(nc, in_maps, core_ids=list(range(NCORES)))
    LAST_RESULT = res

    outp = np.empty((B, L, D), dtype=np.float32)
    for b in range(B):
        acc = res.results[2 * b]["out"].T + res.results[2 * b + 1]["out"].T
        outp[b] = acc + bo
    return outp
